# revision 15
# baseline (speedup 1.0000x reference)
"""Trainium2 Bass kernel for nn_Cross_Encoder (dense cross-transformer).

Sharding: data-parallel over batch B=8 across 8 NeuronCores (1 batch elem/core,
all params replicated). Exploits the rank-1 structure of the edge embedding:
edge_emb[b,i,j,:] = scaled[b,i,j]*edgeW + edgeB, so the relative-bias edge
score collapses to es[b,h,i,j] = scaled[i,j]*qw[h,i] + qb[h,i] with
qw = SCALE*q@edgeW_h, qb = SCALE*q@edgeB_h -- no (B,N,N,E) tensor is ever
materialized.
"""
import math
import numpy as np
from contextlib import ExitStack

import concourse.bass as bass
import concourse.tile as tile
from concourse import bacc, mybir
from concourse import bass_utils
from concourse.masks import make_identity

F32 = mybir.dt.float32
AF = mybir.ActivationFunctionType
OP = mybir.AluOpType

# Model dims (hardcoded per problem spec)
L, E, H, D, FF, MS = 5, 256, 16, 16, 512, 16
B, N = 8, 200
SCALE = 1.0 / math.sqrt(D)
EPS = 1e-5
NCH = [(0, 128), (128, 72)]  # i/j chunking of N=200 into partitions


def _prep_weights(inp):
    """Host-side reshapes/padding of weights into DRAM layouts."""
    w = {}
    g = lambda k: np.asarray(inp[k], dtype=np.float32)
    Wq, Wk, Wv, Wc = g('Wq'), g('Wk'), g('Wv'), g('Wc')
    W1, W2 = g('W1'), g('W2')
    edgeW, edgeB = g('edgeW'), g('edgeB')
    for l in range(L):
        for s in range(2):
            t = f"{l}{s}"
            wqp = np.zeros((E, 2 * E), np.float32)
            wkp = np.zeros((E, 2 * E), np.float32)
            for h in range(H):
                wqp[:, 32 * h:32 * h + 16] = Wq[l, s][:, 16 * h:16 * h + 16] * SCALE
                wkp[:, 32 * h:32 * h + 16] = Wk[l, s][:, 16 * h:16 * h + 16]
            w[f"wq{t}"] = wqp
            w[f"wk{t}"] = wkp
            w[f"wv{t}"] = Wv[l, s]
            wcp = np.zeros((2 * E, E), np.float32)
            for h in range(H):
                wcp[32 * h:32 * h + 16, :] = Wc[l, s][16 * h:16 * h + 16, :]
            w[f"wc{t}"] = wcp
            w[f"w1{t}"] = W1[l, s]
            w[f"w2{t}"] = W2[l, s]
            cols = {}
            cols['bc'] = g('bc')[l, s]
            cols['g1'] = g('g1')[l, s]
            cols['b1'] = g('b1')[l, s]
            cols['g2'] = g('g2')[l, s]
            cols['b2'] = g('b2')[l, s]
            cols['bw2'] = g('bW2')[l, s]
            w[f"cols{t}"] = np.stack([cols[k] for k in
                                      ('bc', 'g1', 'b1', 'g2', 'b2', 'bw2')], axis=1)  # (256,6)
            w[f"bw1{t}"] = g('bW1')[l, s].reshape(FF // 128, 128).T.copy()  # (128,4)
    # edgeW/edgeB block layout for qw/qb matmuls: (128, 32), per head-tile t:
    # cols [8t..8t+4) = edgeW for heads 4t..4t+4, cols [8t+4..8t+8) = edgeB
    ewb = np.zeros((128, 32), np.float32)
    for hh in range(H):
        t, s = hh // 4, hh % 4
        for d in range(16):
            ewb[32 * s + d, 8 * t + s] = edgeW[16 * hh + d]
            ewb[32 * s + d, 8 * t + 4 + s] = edgeB[16 * hh + d]
    w["ewb"] = ewb
    w["nodewb"] = np.stack([g('nodeW'), g('nodeB')], axis=1)  # (256,2)
    # activation-bias constants: gamma[l,s,h,m] grid + EPS (broadcast to rows)
    gam = np.asarray(inp['m1b'], np.float32).reshape(1, -1)  # (1, L*2*H*MS)
    w["gam"] = np.concatenate([gam, np.full((1, 1), EPS, np.float32)], axis=1)
    return w


def _mlp_consts(inp):
    """Per (l,s,h,m) scalar constants for the mixed-score MLP, baked as imms."""
    m1w = np.asarray(inp['m1w'], np.float64)  # (L,2,H,2,MS)
    m1b = np.asarray(inp['m1b'], np.float64)  # (L,2,H,MS)
    m2w = np.asarray(inp['m2w'], np.float64)  # (L,2,H,MS)
    m2b = np.asarray(inp['m2b'], np.float64)  # (L,2,H)
    return m1w, m1b, m2w, m2b


def build(w, mlpc, nlayers=L):
    m1w, m1b, m2w, m2b = mlpc
    nc = bacc.Bacc("TRN2", target_bir_lowering=False, debug=False, num_devices=8)

    # --- DRAM I/O ---
    d_data = nc.dram_tensor("data", [N, N], F32, kind="ExternalInput").ap()
    d_nr = nc.dram_tensor("nr", [1, N], F32, kind="ExternalInput").ap()
    dw = {k: nc.dram_tensor(k, list(v.shape), F32, kind="ExternalInput").ap()
          for k, v in w.items()}
    d_orow = nc.dram_tensor("orow", [E, N], F32, kind="ExternalOutput").ap()
    d_ocol = nc.dram_tensor("ocol", [E, N], F32, kind="ExternalOutput").ap()
    d_scr = nc.dram_tensor("scratch", [1, 2], F32).ap()  # min/max roundtrip

    with tile.TileContext(nc) as tc, ExitStack() as ctx:
        cst = ctx.enter_context(tc.tile_pool(name="cst", bufs=1))
        wp = ctx.enter_context(tc.tile_pool(name="wp", bufs=2))
        xp = ctx.enter_context(tc.tile_pool(name="xp", bufs=2))
        qk = ctx.enter_context(tc.tile_pool(name="qk", bufs=2))
        hp = ctx.enter_context(tc.tile_pool(name="hp", bufs=3))
        sp = ctx.enter_context(tc.tile_pool(name="sp", bufs=2))
        ps = ctx.enter_context(tc.tile_pool(name="ps", bufs=3, space="PSUM"))
        ps2 = ctx.enter_context(tc.tile_pool(name="ps2", bufs=2, space="PSUM"))

        ident = cst.tile([128, 128], F32)
        make_identity(nc, ident[:])

        # ---- load data, compute scaled = (data-mn)/rng, and its transpose ----
        dt_ = [cst.tile([128, N], F32, tag=f"dt{c}", name=f"dt{c}") for c in range(2)]
        for c, (o, p) in enumerate(NCH):
            nc.sync.dma_start(dt_[c][:p, :], d_data[o:o + p, :])
        # per-chunk row max/min -> combine columns -> transpose -> scalar
        ext = cst.tile([128, 4], F32)  # cols: mx c0, mx c1, mn c0, mn c1
        nc.vector.memset(ext[:, 0:2], -3.0e38)
        nc.vector.memset(ext[:, 2:4], 3.0e38)
        for c, (o, p) in enumerate(NCH):
            nc.vector.tensor_reduce(out=ext[:p, c:c + 1], in_=dt_[c][:p, :],
                                    op=OP.max, axis=mybir.AxisListType.X)
            nc.vector.tensor_reduce(out=ext[:p, 2 + c:3 + c], in_=dt_[c][:p, :],
                                    op=OP.min, axis=mybir.AxisListType.X)
        ext2 = cst.tile([128, 2], F32)  # col0 = per-part max, col1 = per-part min
        nc.vector.tensor_tensor(out=ext2[:, 0:1], in0=ext[:, 0:1], in1=ext[:, 1:2],
                                op=OP.max)
        nc.vector.tensor_tensor(out=ext2[:, 1:2], in0=ext[:, 2:3], in1=ext[:, 3:4],
                                op=OP.min)
        extT = cst.tile([1, 256], F32)  # [0:128]=max vals, [128:256]=min vals
        for c2 in range(2):
            tps0 = ps2.tile([1, 128], F32, tag="tp", name="tps0", bufs=2)
            nc.tensor.transpose(tps0[:], ext2[:, c2:c2 + 1], ident[:])
            nc.scalar.copy(extT[:, 128 * c2:128 * c2 + 128], tps0[:])
        fin = cst.tile([1, 2], F32)  # [mx, mn]
        nc.vector.tensor_reduce(out=fin[:, 0:1], in_=extT[:, 0:128], op=OP.max,
                                axis=mybir.AxisListType.X)
        nc.vector.tensor_reduce(out=fin[:, 1:2], in_=extT[:, 128:256], op=OP.min,
                                axis=mybir.AxisListType.X)
        nc.sync.dma_start(d_scr, fin[:])
        mxmn = cst.tile([128, 2], F32)
        nc.sync.dma_start(mxmn[:], d_scr[0:1, :].broadcast_to((128, 2)))
        rng = cst.tile([128, 1], F32)
        nc.vector.tensor_tensor(out=rng[:], in0=mxmn[:, 0:1], in1=mxmn[:, 1:2],
                                op=OP.subtract)
        rinv = cst.tile([128, 1], F32)
        nc.vector.reciprocal(rinv[:], rng[:])
        # s = (data - mn) * rinv ; per chunk
        st_ = [cst.tile([128, N], F32, tag=f"st{c}", name=f"st{c}") for c in range(2)]
        for c, (o, p) in enumerate(NCH):
            nc.vector.tensor_scalar(out=st_[c][:p, :], in0=dt_[c][:p, :],
                                    scalar1=mxmn[:p, 1:2], scalar2=rinv[:p, :],
                                    op0=OP.subtract, op1=OP.mult)
        # transposed scaled (for col blocks)
        sT_ = [cst.tile([128, N], F32, tag=f"sT{c}", name=f"sT{c}") for c in range(2)]
        for cj, (oj, pj) in enumerate(NCH):
            for ci, (oi, pi) in enumerate(NCH):
                tp = ps2.tile([128, 128], F32, tag="tp", name="tps", bufs=2)
                nc.tensor.transpose(tp[:pj, :pi], st_[ci][:pi, oj:oj + pj],
                                    ident[0:pi, 0:pi])
                nc.scalar.copy(sT_[cj][:pj, oi:oi + pi], tp[:pj, :pi])

        # ---- node embedding: xT[e,n] = nodeW[e]*nr[n] + nodeB[e] ----
        nrb = cst.tile([128, N], F32)
        nc.sync.dma_start(nrb[:], d_nr[0:1, :].broadcast_to((128, N)))
        nwb = cst.tile([128, 4], F32)  # [nodeW ec, nodeB ec] x 2 chunks
        nc.sync.dma_start(nwb[:, 0:2], dw["nodewb"][0:128, :])
        nc.sync.dma_start(nwb[:, 2:4], dw["nodewb"][128:256, :])
        xrow = [xp.tile([128, N], F32, tag=f"xr{ec}", name=f"xr{ec}") for ec in range(2)]
        xcol = [xp.tile([128, N], F32, tag=f"xc{ec}", name=f"xc{ec}") for ec in range(2)]
        for ec in range(2):
            nc.vector.tensor_scalar(out=xrow[ec][:], in0=nrb[:],
                                    scalar1=nwb[:, 2 * ec:2 * ec + 1],
                                    scalar2=nwb[:, 2 * ec + 1:2 * ec + 2],
                                    op0=OP.mult, op1=OP.add)
            nc.vector.tensor_copy(xcol[ec][:], xrow[ec][:])

        ewb_sb = cst.tile([128, 32], F32)
        nc.sync.dma_start(ewb_sb[:], dw["ewb"])
        ngam = w["gam"].shape[1]
        gam_sb = cst.tile([128, ngam], F32)
        nc.sync.dma_start(gam_sb[:], dw["gam"][0:1, :].broadcast_to((128, ngam)))
        eps_col = gam_sb[:, ngam - 1:ngam]

        # =================== transformer blocks ===================
        def block(l, s, xq, xkv, stiles):
            """One encoder block; returns new stream tiles (2x (128,N) e-chunks)."""
            t = f"{l}{s}"
            # -- stream weights in --
            wq_sb = [wp.tile([128, 512], F32, tag=f"wq{ec}", name=f"wq{ec}") for ec in range(2)]
            wk_sb = [wp.tile([128, 512], F32, tag=f"wk{ec}", name=f"wk{ec}") for ec in range(2)]
            wv_sb = [wp.tile([128, 256], F32, tag=f"wv{ec}", name=f"wv{ec}") for ec in range(2)]
            wc_sb = [wp.tile([128, 256], F32, tag=f"wc{m}", name=f"wc{m}") for m in range(4)]
            w1_sb = [wp.tile([128, 512], F32, tag=f"w1{ec}", name=f"w1{ec}") for ec in range(2)]
            w2_sb = [wp.tile([128, 256], F32, tag=f"w2{m}", name=f"w2{m}") for m in range(4)]
            for ec in range(2):
                nc.sync.dma_start(wq_sb[ec][:], dw[f"wq{t}"][128 * ec:128 * ec + 128, :])
                nc.sync.dma_start(wk_sb[ec][:], dw[f"wk{t}"][128 * ec:128 * ec + 128, :])
                nc.sync.dma_start(wv_sb[ec][:], dw[f"wv{t}"][128 * ec:128 * ec + 128, :])
                nc.sync.dma_start(w1_sb[ec][:], dw[f"w1{t}"][128 * ec:128 * ec + 128, :])
            for m in range(4):
                nc.sync.dma_start(wc_sb[m][:], dw[f"wc{t}"][128 * m:128 * m + 128, :])
                nc.sync.dma_start(w2_sb[m][:], dw[f"w2{t}"][128 * m:128 * m + 128, :])
            colw = wp.tile([128, 12], F32, tag="colw", name="colw")  # 6 cols x 2 e-chunks
            nc.sync.dma_start(colw[:, 0:6], dw[f"cols{t}"][0:128, :])
            nc.sync.dma_start(colw[:, 6:12], dw[f"cols{t}"][128:256, :])
            bw1_sb = wp.tile([128, 4], F32, tag="bw1", name="bw1")
            nc.sync.dma_start(bw1_sb[:], dw[f"bw1{t}"])

            def ccol(name, ec):
                i = ('bc', 'g1', 'b1', 'g2', 'b2', 'bw2').index(name)
                return colw[:, 6 * ec + i:6 * ec + i + 1]

            # -- q/k projections (padded head layout), v natural --
            qT = [qk.tile([128, N], F32, tag=f"qT{m}", name=f"qT{m}") for m in range(4)]
            kT = [qk.tile([128, N], F32, tag=f"kT{m}", name=f"kT{m}") for m in range(4)]
            for m in range(4):
                pq = ps.tile([128, N], F32, tag="mm", name="pq", bufs=3)
                pk = ps.tile([128, N], F32, tag="mm", name="pk", bufs=3)
                for ec in range(2):
                    nc.tensor.matmul(pq[:], wq_sb[ec][:, 128 * m:128 * m + 128],
                                     xq[ec][:], start=(ec == 0), stop=(ec == 1))
                for ec in range(2):
                    nc.tensor.matmul(pk[:], wk_sb[ec][:, 128 * m:128 * m + 128],
                                     xkv[ec][:], start=(ec == 0), stop=(ec == 1))
                nc.scalar.copy(qT[m][:], pq[:])
                nc.scalar.copy(kT[m][:], pk[:])
            v_sb = [qk.tile([128, 256], F32, tag=f"v{c}", name=f"v{c}") for c in range(2)]
            for c, (o, p) in enumerate(NCH):
                pv = ps.tile([128, 256], F32, tag="mm", name="pv", bufs=3)
                for ec in range(2):
                    nc.tensor.matmul(pv[:p, :], xkv[ec][:, o:o + p], wv_sb[ec][:],
                                     start=(ec == 0), stop=(ec == 1))
                nc.scalar.copy(v_sb[c][:p, :], pv[:p, :])

            # -- qw/qb: per head-tile m, per i-chunk --
            qwb = [qk.tile([128, 32], F32, tag=f"qwb{c}", name=f"qwb{c}") for c in range(2)]
            for c, (o, p) in enumerate(NCH):
                for m in range(4):
                    pw = ps2.tile([128, 8], F32, tag="pw", name="pw", bufs=2)
                    nc.tensor.matmul(pw[:p, :], qT[m][:, o:o + p],
                                     ewb_sb[:, 8 * m:8 * m + 8], start=True, stop=True)
                    nc.scalar.copy(qwb[c][:p, 8 * m:8 * m + 8], pw[:p, :])

            # -- per-head attention --
            avT = [hp.tile([128, N], F32, tag=f"avT{m}", name=f"avT{m}") for m in range(4)]
            for h in range(H):
                ht, hs = h // 4, h % 4
                dot = [hp.tile([128, N], F32, tag=f"dot{c}", name=f"dot{c}") for c in range(2)]
                es = [hp.tile([128, N], F32, tag=f"es{c}", name=f"es{c}") for c in range(2)]
                for c, (o, p) in enumerate(NCH):
                    pd = ps.tile([128, N], F32, tag="mm", name="pd", bufs=3)
                    nc.tensor.matmul(pd[:p, :],
                                     qT[ht][32 * hs:32 * hs + 16, o:o + p],
                                     kT[ht][32 * hs:32 * hs + 16, :],
                                     start=True, stop=True,
                                     tile_position=(32 * hs, 0))
                    nc.scalar.copy(dot[c][:p, :], pd[:p, :])
                    nc.vector.tensor_scalar(
                        out=es[c][:p, :], in0=stiles[c][:p, :],
                        scalar1=qwb[c][:p, 8 * ht + hs:8 * ht + hs + 1],
                        scalar2=qwb[c][:p, 8 * ht + 4 + hs:8 * ht + 5 + hs],
                        op0=OP.mult, op1=OP.add)
                # mixed-score MLP: acc = sum_m c_m relu(a_m*dot + b_m*es + g_m) + m2b
                acc = [hp.tile([128, N], F32, tag=f"acc{c}", name=f"acc{c}") for c in range(2)]
                tmp = [hp.tile([128, N], F32, tag=f"tmp{c}", name=f"tmp{c}") for c in range(2)]
                for c, (o, p) in enumerate(NCH):
                    nc.vector.memset(acc[c][:p, :], float(m2b[l, s, h]))
                for m in range(MS):
                    al = float(m1w[l, s, h, 0, m]); be = float(m1w[l, s, h, 1, m])
                    c2 = float(m2w[l, s, h, m])
                    gidx = ((l * 2 + s) * H + h) * MS + m
                    gcol = gam_sb[:, gidx:gidx + 1]
                    for c, (o, p) in enumerate(NCH):
                        if abs(al) >= 1e-6:
                            nc.vector.scalar_tensor_tensor(
                                out=tmp[c][:p, :], in0=es[c][:p, :],
                                scalar=be / abs(al), in1=dot[c][:p, :],
                                op0=OP.mult,
                                op1=(OP.add if al > 0 else OP.subtract))
                            nc.scalar.activation(tmp[c][:p, :], tmp[c][:p, :],
                                                 AF.Relu, bias=gcol[:p, :],
                                                 scale=abs(al))
                        else:
                            nc.vector.tensor_scalar(out=tmp[c][:p, :],
                                                    in0=es[c][:p, :], scalar1=be,
                                                    scalar2=None, op0=OP.mult)
                            nc.scalar.activation(tmp[c][:p, :], tmp[c][:p, :],
                                                 AF.Relu, bias=gcol[:p, :])
                        nc.vector.scalar_tensor_tensor(
                            out=acc[c][:p, :], in0=tmp[c][:p, :], scalar=c2,
                            in1=acc[c][:p, :], op0=OP.mult, op1=OP.add)
                # softmax over j (free axis), normalized before transpose
                pex = [hp.tile([128, N], F32, tag=f"pex{c}", name=f"pex{c}") for c in range(2)]
                for c, (o, p) in enumerate(NCH):
                    nmx = hp.tile([128, 1], F32, tag=f"nmx{c}", name=f"nmx{c}")
                    rs = hp.tile([128, 1], F32, tag=f"rs{c}", name=f"rs{c}")
                    nc.vector.tensor_reduce(out=nmx[:p, :], in_=acc[c][:p, :],
                                            op=OP.max, axis=mybir.AxisListType.X,
                                            negate=True)
                    nc.scalar.activation(pex[c][:p, :], acc[c][:p, :], AF.Exp,
                                         bias=nmx[:p, :], scale=1.0,
                                         accum_out=rs[:p, :])
                    nc.vector.reciprocal(rs[:p, :], rs[:p, :])
                    nc.vector.tensor_scalar(out=pex[c][:p, :], in0=pex[c][:p, :],
                                            scalar1=rs[:p, :], scalar2=None,
                                            op0=OP.mult)
                # transpose attn -> pT (j-part, i-free)
                pT = [hp.tile([128, N], F32, tag=f"pT{c}", name=f"pT{c}") for c in range(2)]
                for cj, (oj, pj) in enumerate(NCH):
                    for ci, (oi, pi) in enumerate(NCH):
                        tp = ps2.tile([128, 128], F32, tag="tp", name="tpa", bufs=2)
                        nc.tensor.transpose(tp[:pj, :pi],
                                            pex[ci][:pi, oj:oj + pj],
                                            ident[0:pi, 0:pi])
                        nc.scalar.copy(pT[cj][:pj, oi:oi + pi], tp[:pj, :pi])
                # attn @ v -> avT rows [32hs:32hs+16] of head-tile ht
                pav = ps.tile([128, N], F32, tag="mm", name="pav", bufs=3)
                for cj, (oj, pj) in enumerate(NCH):
                    nc.tensor.matmul(pav[32 * hs:32 * hs + 16, :],
                                     v_sb[cj][:pj, 16 * h:16 * h + 16],
                                     pT[cj][:pj, :],
                                     start=(cj == 0), stop=(cj == 1),
                                     tile_position=(0, 32 * hs))
                nc.scalar.copy(avT[ht][32 * hs:32 * hs + 16, :],
                               pav[32 * hs:32 * hs + 16, :])

            # -- combine + residual + instance norm --
            def inorm(xsum, ec, gname, bname, out_tile):
                """out = IN(xsum) with affine g,b. xsum: SBUF (128,N)."""
                sm = sp.tile([128, 1], F32, tag="sm", name="sm")
                sq = sp.tile([128, 1], F32, tag="sq", name="sq")
                sqd = sp.tile([128, N], F32, tag="sqd", name="sqd")
                nc.vector.tensor_reduce(out=sm[:], in_=xsum[:], op=OP.add,
                                        axis=mybir.AxisListType.X)
                nc.scalar.activation(sqd[:], xsum[:], AF.Square, accum_out=sq[:])
                mu = sp.tile([128, 1], F32, tag="mu", name="mu")
                var = sp.tile([128, 1], F32, tag="var", name="var")
                nc.vector.tensor_scalar(out=mu[:], in0=sm[:], scalar1=1.0 / N,
                                        scalar2=None, op0=OP.mult)
                # var = sq/N - mu^2
                musq = sp.tile([128, 1], F32, tag="musq", name="musq")
                nc.vector.tensor_tensor(out=musq[:], in0=mu[:], in1=mu[:], op=OP.mult)
                nc.vector.scalar_tensor_tensor(out=var[:], in0=sq[:], scalar=1.0 / N,
                                               in1=musq[:], op0=OP.mult, op1=OP.subtract)
                std = sp.tile([128, 1], F32, tag="std", name="std")
                nc.scalar.activation(std[:], var[:], AF.Sqrt, bias=eps_col)
                nc.vector.reciprocal(std[:], std[:])
                gs = sp.tile([128, 1], F32, tag="gs", name="gs")
                nc.vector.tensor_tensor(out=gs[:], in0=std[:], in1=ccol(gname, ec),
                                        op=OP.mult)
                nc.vector.tensor_scalar(out=out_tile[:], in0=xsum[:], scalar1=mu[:],
                                        scalar2=gs[:], op0=OP.subtract, op1=OP.mult)
                nc.scalar.activation(out_tile[:], out_tile[:], AF.Identity,
                                     bias=ccol(bname, ec), scale=1.0)

            o1 = [sp.tile([128, N], F32, tag=f"o1{ec}", name=f"o1{ec}") for ec in range(2)]
            for ec in range(2):
                pm = ps.tile([128, N], F32, tag="mm", name="pm", bufs=3)
                for m in range(4):
                    nc.tensor.matmul(pm[:], wc_sb[m][:, 128 * ec:128 * ec + 128],
                                     avT[m][:], start=(m == 0), stop=(m == 3))
                # x + mh + bc -> IN with g1,b1
                xs = sp.tile([128, N], F32, tag="xs", name="xs")
                nc.vector.scalar_tensor_tensor(out=xs[:], in0=pm[:],
                                               scalar=ccol('bc', ec), in1=xq[ec][:],
                                               op0=OP.add, op1=OP.add)
                inorm(xs, ec, 'g1', 'b1', o1[ec])

            # -- FFN --
            h1 = [sp.tile([128, N], F32, tag=f"h1{m}", name=f"h1{m}") for m in range(4)]
            for m in range(4):
                ph = ps.tile([128, N], F32, tag="mm", name="ph", bufs=3)
                for ec in range(2):
                    nc.tensor.matmul(ph[:], w1_sb[ec][:, 128 * m:128 * m + 128],
                                     o1[ec][:], start=(ec == 0), stop=(ec == 1))
                nc.scalar.activation(h1[m][:], ph[:], AF.Relu,
                                     bias=bw1_sb[:, m:m + 1], scale=1.0)
            xnew = [xp.tile([128, N], F32, tag=f"xn{s}{ec}", name=f"xn{s}{ec}") for ec in range(2)]
            for ec in range(2):
                po = ps.tile([128, N], F32, tag="mm", name="po", bufs=3)
                for m in range(4):
                    nc.tensor.matmul(po[:], w2_sb[m][:, 128 * ec:128 * ec + 128],
                                     h1[m][:], start=(m == 0), stop=(m == 3))
                xs2 = sp.tile([128, N], F32, tag="xs2", name="xs2")
                nc.vector.scalar_tensor_tensor(out=xs2[:], in0=po[:],
                                               scalar=ccol('bw2', ec), in1=o1[ec][:],
                                               op0=OP.add, op1=OP.add)
                inorm(xs2, ec, 'g2', 'b2', xnew[ec])
            return xnew

        for l in range(nlayers):
            xrow_new = block(l, 0, xrow, xcol, st_)
            xcol_new = block(l, 1, xcol, xrow, sT_)
            xrow, xcol = xrow_new, xcol_new

        for ec in range(2):
            nc.sync.dma_start(d_orow[128 * ec:128 * ec + 128, :], xrow[ec][:])
            nc.sync.dma_start(d_ocol[128 * ec:128 * ec + 128, :], xcol[ec][:])

    nc.compile()
    return nc


LAST_EXEC_NS = [None]


def make_runner(inputs):
    """Build the bass program + a reusable jitted multi-core executor.

    Returns run() -> (row, col) full-batch outputs."""
    import jax
    from jax.sharding import Mesh, PartitionSpec
    from jax.experimental.shard_map import shard_map
    from concourse import bass2jax, mybir as _mb

    w = _prep_weights(inputs)
    mlpc = _mlp_consts(inputs)
    nc = build(w, mlpc)
    data = np.asarray(inputs['data'], np.float32)
    nr = np.asarray(inputs['node_rand'], np.float32)
    in_maps = []
    for b in range(B):
        m = {"data": data[b], "nr": nr[b].reshape(1, N)}
        m.update(w)
        in_maps.append(m)

    bass2jax.install_neuronx_cc_hook()
    partition_name = nc.partition_id_tensor.name if nc.partition_id_tensor else None
    in_names, out_names, out_avals, zero_outs = [], [], [], []
    for alloc in nc.m.functions[0].allocations:
        if not isinstance(alloc, _mb.MemoryLocationSet):
            continue
        name = alloc.memorylocations[0].name
        if alloc.kind == "ExternalInput":
            if name != partition_name:
                in_names.append(name)
        elif alloc.kind == "ExternalOutput":
            shape = tuple(alloc.tensor_shape)
            dtype = _mb.dt.np(alloc.dtype)
            out_names.append(name)
            out_avals.append(jax.core.ShapedArray(shape, dtype))
            zero_outs.append(np.zeros(shape, dtype))
    n_params = len(in_names)
    n_outs = len(out_avals)
    all_names = in_names + out_names + ([partition_name] if partition_name else [])
    donate = tuple(range(n_params, n_params + n_outs))

    def _body(*args):
        operands = list(args)
        if partition_name is not None:
            operands.append(bass2jax.partition_id_tensor())
        outs = bass2jax._bass_exec_p.bind(
            *operands, out_avals=tuple(out_avals), in_names=tuple(all_names),
            out_names=tuple(out_names), lowering_input_output_aliases=(),
            sim_require_finite=True, sim_require_nnan=True, nc=nc)
        return tuple(outs)

    devices = jax.devices()[:B]
    mesh = Mesh(np.asarray(devices), ("core",))
    sharded = jax.jit(
        shard_map(_body, mesh=mesh,
                  in_specs=(PartitionSpec("core"),) * (n_params + n_outs),
                  out_specs=(PartitionSpec("core"),) * n_outs,
                  check_rep=False),
        donate_argnums=donate, keep_unused=True)
    concat_in = [np.concatenate([np.asarray(in_maps[c][nm]) for c in range(B)], axis=0)
                 for nm in in_names]

    def run():
        zo = [np.concatenate([z] * B, axis=0) for z in zero_outs]
        outs = sharded(*concat_in, *zo)
        outs = [np.asarray(o) for o in outs]
        res = {}
        for i, nm in enumerate(out_names):
            per = np.split(outs[i], B, axis=0)
            res[nm] = per
        row = np.stack([res["orow"][b].T for b in range(B)])
        col = np.stack([res["ocol"][b].T for b in range(B)])
        return row.astype(np.float32), col.astype(np.float32)

    return run


def kernel(**inputs):
    run = make_runner(inputs)
    return run()


# revision 16
# speedup vs baseline: 13.4053x; 13.4053x over previous
"""Trainium2 Bass kernel for nn_Cross_Encoder (dense cross-transformer).

Sharding: data-parallel over batch B=8 across 8 NeuronCores (1 batch elem/core,
all params replicated). Exploits the rank-1 structure of the edge embedding:
edge_emb[b,i,j,:] = scaled[b,i,j]*edgeW + edgeB, so the relative-bias edge
score collapses to es[b,h,i,j] = scaled[i,j]*qw[h,i] + qb[h,i] with
qw = SCALE*q@edgeW_h, qb = SCALE*q@edgeB_h -- no (B,N,N,E) tensor is ever
materialized.
"""
import math
import numpy as np
from contextlib import ExitStack

import concourse.bass as bass
import concourse.tile as tile
from concourse import bacc, mybir
from concourse import bass_utils
from concourse.masks import make_identity

F32 = mybir.dt.float32
AF = mybir.ActivationFunctionType
OP = mybir.AluOpType

# Model dims (hardcoded per problem spec)
L, E, H, D, FF, MS = 5, 256, 16, 16, 512, 16
B, N = 8, 200
SCALE = 1.0 / math.sqrt(D)
EPS = 1e-5
NCH = [(0, 128), (128, 72)]  # i/j chunking of N=200 into partitions


def _prep_weights(inp):
    """Host-side reshapes/padding of weights into DRAM layouts."""
    w = {}
    g = lambda k: np.asarray(inp[k], dtype=np.float32)
    Wq, Wk, Wv, Wc = g('Wq'), g('Wk'), g('Wv'), g('Wc')
    W1, W2 = g('W1'), g('W2')
    edgeW, edgeB = g('edgeW'), g('edgeB')
    for l in range(L):
        for s in range(2):
            t = f"{l}{s}"
            wqp = np.zeros((E, 2 * E), np.float32)
            wkp = np.zeros((E, 2 * E), np.float32)
            for h in range(H):
                wqp[:, 32 * h:32 * h + 16] = Wq[l, s][:, 16 * h:16 * h + 16] * SCALE
                wkp[:, 32 * h:32 * h + 16] = Wk[l, s][:, 16 * h:16 * h + 16]
            w[f"wq{t}"] = wqp
            w[f"wk{t}"] = wkp
            w[f"wv{t}"] = Wv[l, s]
            wcp = np.zeros((2 * E, E), np.float32)
            for h in range(H):
                wcp[32 * h:32 * h + 16, :] = Wc[l, s][16 * h:16 * h + 16, :]
            w[f"wc{t}"] = wcp
            w[f"w1{t}"] = W1[l, s]
            w[f"w2{t}"] = W2[l, s]
            cols = {}
            cols['bc'] = g('bc')[l, s]
            cols['g1'] = g('g1')[l, s]
            cols['b1'] = g('b1')[l, s]
            cols['g2'] = g('g2')[l, s]
            cols['b2'] = g('b2')[l, s]
            cols['bw2'] = g('bW2')[l, s]
            w[f"cols{t}"] = np.stack([cols[k] for k in
                                      ('bc', 'g1', 'b1', 'g2', 'b2', 'bw2')], axis=1)  # (256,6)
            w[f"bw1{t}"] = g('bW1')[l, s].reshape(FF // 128, 128).T.copy()  # (128,4)
    # edgeW/edgeB block layout for qw/qb matmuls: (128, 32), per head-tile t:
    # cols [8t..8t+4) = edgeW for heads 4t..4t+4, cols [8t+4..8t+8) = edgeB
    ewb = np.zeros((128, 32), np.float32)
    for hh in range(H):
        t, s = hh // 4, hh % 4
        for d in range(16):
            ewb[32 * s + d, 8 * t + s] = edgeW[16 * hh + d]
            ewb[32 * s + d, 8 * t + 4 + s] = edgeB[16 * hh + d]
    w["ewb"] = ewb
    w["nodewb"] = np.stack([g('nodeW'), g('nodeB')], axis=1)  # (256,2)
    # activation-bias constants: gamma[l,s,h,m] grid + EPS (broadcast to rows)
    gam = np.asarray(inp['m1b'], np.float32).reshape(1, -1)  # (1, L*2*H*MS)
    w["gam"] = np.concatenate([gam, np.full((1, 1), EPS, np.float32)], axis=1)
    return w


def _mlp_consts(inp):
    """Per (l,s,h,m) scalar constants for the mixed-score MLP, baked as imms."""
    m1w = np.asarray(inp['m1w'], np.float64)  # (L,2,H,2,MS)
    m1b = np.asarray(inp['m1b'], np.float64)  # (L,2,H,MS)
    m2w = np.asarray(inp['m2w'], np.float64)  # (L,2,H,MS)
    m2b = np.asarray(inp['m2b'], np.float64)  # (L,2,H)
    return m1w, m1b, m2w, m2b


def build(w, mlpc, nlayers=L):
    m1w, m1b, m2w, m2b = mlpc
    nc = bacc.Bacc("TRN2", target_bir_lowering=False, debug=False, num_devices=8)

    # --- DRAM I/O ---
    d_data = nc.dram_tensor("data", [N, N], F32, kind="ExternalInput").ap()
    d_nr = nc.dram_tensor("nr", [1, N], F32, kind="ExternalInput").ap()
    dw = {k: nc.dram_tensor(k, list(v.shape), F32, kind="ExternalInput").ap()
          for k, v in w.items()}
    d_orow = nc.dram_tensor("orow", [E, N], F32, kind="ExternalOutput").ap()
    d_ocol = nc.dram_tensor("ocol", [E, N], F32, kind="ExternalOutput").ap()
    d_scr = nc.dram_tensor("scratch", [1, 2], F32).ap()  # min/max roundtrip

    with tile.TileContext(nc) as tc, ExitStack() as ctx:
        cst = ctx.enter_context(tc.tile_pool(name="cst", bufs=1))
        wp = ctx.enter_context(tc.tile_pool(name="wp", bufs=2))
        xp = ctx.enter_context(tc.tile_pool(name="xp", bufs=2))
        qk = ctx.enter_context(tc.tile_pool(name="qk", bufs=2))
        hp = ctx.enter_context(tc.tile_pool(name="hp", bufs=3))
        sp = ctx.enter_context(tc.tile_pool(name="sp", bufs=2))
        ps = ctx.enter_context(tc.tile_pool(name="ps", bufs=3, space="PSUM"))
        ps2 = ctx.enter_context(tc.tile_pool(name="ps2", bufs=2, space="PSUM"))

        ident = cst.tile([128, 128], F32)
        make_identity(nc, ident[:])

        # ---- load data, compute scaled = (data-mn)/rng, and its transpose ----
        dt_ = [cst.tile([128, N], F32, tag=f"dt{c}", name=f"dt{c}") for c in range(2)]
        for c, (o, p) in enumerate(NCH):
            nc.sync.dma_start(dt_[c][:p, :], d_data[o:o + p, :])
        # per-chunk row max/min -> combine columns -> transpose -> scalar
        ext = cst.tile([128, 4], F32)  # cols: mx c0, mx c1, mn c0, mn c1
        nc.vector.memset(ext[:, 0:2], -3.0e38)
        nc.vector.memset(ext[:, 2:4], 3.0e38)
        for c, (o, p) in enumerate(NCH):
            nc.vector.tensor_reduce(out=ext[:p, c:c + 1], in_=dt_[c][:p, :],
                                    op=OP.max, axis=mybir.AxisListType.X)
            nc.vector.tensor_reduce(out=ext[:p, 2 + c:3 + c], in_=dt_[c][:p, :],
                                    op=OP.min, axis=mybir.AxisListType.X)
        ext2 = cst.tile([128, 2], F32)  # col0 = per-part max, col1 = per-part min
        nc.vector.tensor_tensor(out=ext2[:, 0:1], in0=ext[:, 0:1], in1=ext[:, 1:2],
                                op=OP.max)
        nc.vector.tensor_tensor(out=ext2[:, 1:2], in0=ext[:, 2:3], in1=ext[:, 3:4],
                                op=OP.min)
        extT = cst.tile([1, 256], F32)  # [0:128]=max vals, [128:256]=min vals
        for c2 in range(2):
            tps0 = ps2.tile([1, 128], F32, tag="tp", name="tps0", bufs=2)
            nc.tensor.transpose(tps0[:], ext2[:, c2:c2 + 1], ident[:])
            nc.scalar.copy(extT[:, 128 * c2:128 * c2 + 128], tps0[:])
        fin = cst.tile([1, 2], F32)  # [mx, mn]
        nc.vector.tensor_reduce(out=fin[:, 0:1], in_=extT[:, 0:128], op=OP.max,
                                axis=mybir.AxisListType.X)
        nc.vector.tensor_reduce(out=fin[:, 1:2], in_=extT[:, 128:256], op=OP.min,
                                axis=mybir.AxisListType.X)
        nc.sync.dma_start(d_scr, fin[:])
        mxmn = cst.tile([128, 2], F32)
        nc.sync.dma_start(mxmn[:], d_scr[0:1, :].broadcast_to((128, 2)))
        rng = cst.tile([128, 1], F32)
        nc.vector.tensor_tensor(out=rng[:], in0=mxmn[:, 0:1], in1=mxmn[:, 1:2],
                                op=OP.subtract)
        rinv = cst.tile([128, 1], F32)
        nc.vector.reciprocal(rinv[:], rng[:])
        # s = (data - mn) * rinv ; per chunk
        st_ = [cst.tile([128, N], F32, tag=f"st{c}", name=f"st{c}") for c in range(2)]
        for c, (o, p) in enumerate(NCH):
            nc.vector.tensor_scalar(out=st_[c][:p, :], in0=dt_[c][:p, :],
                                    scalar1=mxmn[:p, 1:2], scalar2=rinv[:p, :],
                                    op0=OP.subtract, op1=OP.mult)
        # transposed scaled (for col blocks)
        sT_ = [cst.tile([128, N], F32, tag=f"sT{c}", name=f"sT{c}") for c in range(2)]
        for cj, (oj, pj) in enumerate(NCH):
            for ci, (oi, pi) in enumerate(NCH):
                tp = ps2.tile([128, 128], F32, tag="tp", name="tps", bufs=2)
                nc.tensor.transpose(tp[:pj, :pi], st_[ci][:pi, oj:oj + pj],
                                    ident[0:pi, 0:pi])
                nc.scalar.copy(sT_[cj][:pj, oi:oi + pi], tp[:pj, :pi])

        # ---- node embedding: xT[e,n] = nodeW[e]*nr[n] + nodeB[e] ----
        nrb = cst.tile([128, N], F32)
        nc.sync.dma_start(nrb[:], d_nr[0:1, :].broadcast_to((128, N)))
        nwb = cst.tile([128, 4], F32)  # [nodeW ec, nodeB ec] x 2 chunks
        nc.sync.dma_start(nwb[:, 0:2], dw["nodewb"][0:128, :])
        nc.sync.dma_start(nwb[:, 2:4], dw["nodewb"][128:256, :])
        xrow = [xp.tile([128, N], F32, tag=f"xr{ec}", name=f"xr{ec}") for ec in range(2)]
        xcol = [xp.tile([128, N], F32, tag=f"xc{ec}", name=f"xc{ec}") for ec in range(2)]
        for ec in range(2):
            nc.vector.tensor_scalar(out=xrow[ec][:], in0=nrb[:],
                                    scalar1=nwb[:, 2 * ec:2 * ec + 1],
                                    scalar2=nwb[:, 2 * ec + 1:2 * ec + 2],
                                    op0=OP.mult, op1=OP.add)
            nc.vector.tensor_copy(xcol[ec][:], xrow[ec][:])

        ewb_sb = cst.tile([128, 32], F32)
        nc.sync.dma_start(ewb_sb[:], dw["ewb"])
        ngam = w["gam"].shape[1]
        gam_sb = cst.tile([128, ngam], F32)
        nc.sync.dma_start(gam_sb[:], dw["gam"][0:1, :].broadcast_to((128, ngam)))
        eps_col = gam_sb[:, ngam - 1:ngam]

        # =================== transformer blocks ===================
        def block(l, s, xq, xkv, stiles):
            """One encoder block; returns new stream tiles (2x (128,N) e-chunks)."""
            t = f"{l}{s}"
            # -- stream weights in --
            wq_sb = [wp.tile([128, 512], F32, tag=f"wq{ec}", name=f"wq{ec}") for ec in range(2)]
            wk_sb = [wp.tile([128, 512], F32, tag=f"wk{ec}", name=f"wk{ec}") for ec in range(2)]
            wv_sb = [wp.tile([128, 256], F32, tag=f"wv{ec}", name=f"wv{ec}") for ec in range(2)]
            wc_sb = [wp.tile([128, 256], F32, tag=f"wc{m}", name=f"wc{m}") for m in range(4)]
            w1_sb = [wp.tile([128, 512], F32, tag=f"w1{ec}", name=f"w1{ec}") for ec in range(2)]
            w2_sb = [wp.tile([128, 256], F32, tag=f"w2{m}", name=f"w2{m}") for m in range(4)]
            for ec in range(2):
                nc.sync.dma_start(wq_sb[ec][:], dw[f"wq{t}"][128 * ec:128 * ec + 128, :])
                nc.sync.dma_start(wk_sb[ec][:], dw[f"wk{t}"][128 * ec:128 * ec + 128, :])
                nc.sync.dma_start(wv_sb[ec][:], dw[f"wv{t}"][128 * ec:128 * ec + 128, :])
                nc.sync.dma_start(w1_sb[ec][:], dw[f"w1{t}"][128 * ec:128 * ec + 128, :])
            for m in range(4):
                nc.sync.dma_start(wc_sb[m][:], dw[f"wc{t}"][128 * m:128 * m + 128, :])
                nc.sync.dma_start(w2_sb[m][:], dw[f"w2{t}"][128 * m:128 * m + 128, :])
            colw = wp.tile([128, 12], F32, tag="colw", name="colw")  # 6 cols x 2 e-chunks
            nc.sync.dma_start(colw[:, 0:6], dw[f"cols{t}"][0:128, :])
            nc.sync.dma_start(colw[:, 6:12], dw[f"cols{t}"][128:256, :])
            bw1_sb = wp.tile([128, 4], F32, tag="bw1", name="bw1")
            nc.sync.dma_start(bw1_sb[:], dw[f"bw1{t}"])

            def ccol(name, ec):
                i = ('bc', 'g1', 'b1', 'g2', 'b2', 'bw2').index(name)
                return colw[:, 6 * ec + i:6 * ec + i + 1]

            # -- q/k projections (padded head layout), v natural --
            qT = [qk.tile([128, N], F32, tag=f"qT{m}", name=f"qT{m}") for m in range(4)]
            kT = [qk.tile([128, N], F32, tag=f"kT{m}", name=f"kT{m}") for m in range(4)]
            for m in range(4):
                pq = ps.tile([128, N], F32, tag="mm", name="pq", bufs=3)
                pk = ps.tile([128, N], F32, tag="mm", name="pk", bufs=3)
                for ec in range(2):
                    nc.tensor.matmul(pq[:], wq_sb[ec][:, 128 * m:128 * m + 128],
                                     xq[ec][:], start=(ec == 0), stop=(ec == 1))
                for ec in range(2):
                    nc.tensor.matmul(pk[:], wk_sb[ec][:, 128 * m:128 * m + 128],
                                     xkv[ec][:], start=(ec == 0), stop=(ec == 1))
                nc.scalar.copy(qT[m][:], pq[:])
                nc.scalar.copy(kT[m][:], pk[:])
            v_sb = [qk.tile([128, 256], F32, tag=f"v{c}", name=f"v{c}") for c in range(2)]
            for c, (o, p) in enumerate(NCH):
                pv = ps.tile([128, 256], F32, tag="mm", name="pv", bufs=3)
                for ec in range(2):
                    nc.tensor.matmul(pv[:p, :], xkv[ec][:, o:o + p], wv_sb[ec][:],
                                     start=(ec == 0), stop=(ec == 1))
                nc.scalar.copy(v_sb[c][:p, :], pv[:p, :])

            # -- qw/qb: per head-tile m, per i-chunk --
            qwb = [qk.tile([128, 32], F32, tag=f"qwb{c}", name=f"qwb{c}") for c in range(2)]
            for c, (o, p) in enumerate(NCH):
                for m in range(4):
                    pw = ps2.tile([128, 8], F32, tag="pw", name="pw", bufs=2)
                    nc.tensor.matmul(pw[:p, :], qT[m][:, o:o + p],
                                     ewb_sb[:, 8 * m:8 * m + 8], start=True, stop=True)
                    nc.scalar.copy(qwb[c][:p, 8 * m:8 * m + 8], pw[:p, :])

            # -- per-head attention --
            avT = [hp.tile([128, N], F32, tag=f"avT{m}", name=f"avT{m}") for m in range(4)]
            for h in range(H):
                ht, hs = h // 4, h % 4
                dot = [hp.tile([128, N], F32, tag=f"dot{c}", name=f"dot{c}") for c in range(2)]
                es = [hp.tile([128, N], F32, tag=f"es{c}", name=f"es{c}") for c in range(2)]
                for c, (o, p) in enumerate(NCH):
                    pd = ps.tile([128, N], F32, tag="mm", name="pd", bufs=3)
                    nc.tensor.matmul(pd[:p, :],
                                     qT[ht][32 * hs:32 * hs + 16, o:o + p],
                                     kT[ht][32 * hs:32 * hs + 16, :],
                                     start=True, stop=True,
                                     tile_position=(32 * hs, 0))
                    nc.scalar.copy(dot[c][:p, :], pd[:p, :])
                    nc.vector.tensor_scalar(
                        out=es[c][:p, :], in0=stiles[c][:p, :],
                        scalar1=qwb[c][:p, 8 * ht + hs:8 * ht + hs + 1],
                        scalar2=qwb[c][:p, 8 * ht + 4 + hs:8 * ht + 5 + hs],
                        op0=OP.mult, op1=OP.add)
                # mixed-score MLP: acc = sum_m c_m relu(a_m*dot + b_m*es + g_m) + m2b
                acc = [hp.tile([128, N], F32, tag=f"acc{c}", name=f"acc{c}") for c in range(2)]
                tmp = [hp.tile([128, N], F32, tag=f"tmp{c}", name=f"tmp{c}") for c in range(2)]
                for c, (o, p) in enumerate(NCH):
                    nc.vector.memset(acc[c][:p, :], float(m2b[l, s, h]))
                for m in range(MS):
                    al = float(m1w[l, s, h, 0, m]); be = float(m1w[l, s, h, 1, m])
                    c2 = float(m2w[l, s, h, m])
                    gidx = ((l * 2 + s) * H + h) * MS + m
                    gcol = gam_sb[:, gidx:gidx + 1]
                    for c, (o, p) in enumerate(NCH):
                        if abs(al) >= 1e-6:
                            nc.vector.scalar_tensor_tensor(
                                out=tmp[c][:p, :], in0=es[c][:p, :],
                                scalar=be / abs(al), in1=dot[c][:p, :],
                                op0=OP.mult,
                                op1=(OP.add if al > 0 else OP.subtract))
                            nc.scalar.activation(tmp[c][:p, :], tmp[c][:p, :],
                                                 AF.Relu, bias=gcol[:p, :],
                                                 scale=abs(al))
                        else:
                            nc.vector.tensor_scalar(out=tmp[c][:p, :],
                                                    in0=es[c][:p, :], scalar1=be,
                                                    scalar2=None, op0=OP.mult)
                            nc.scalar.activation(tmp[c][:p, :], tmp[c][:p, :],
                                                 AF.Relu, bias=gcol[:p, :])
                        nc.vector.scalar_tensor_tensor(
                            out=acc[c][:p, :], in0=tmp[c][:p, :], scalar=c2,
                            in1=acc[c][:p, :], op0=OP.mult, op1=OP.add)
                # softmax over j (free axis), normalized before transpose
                pex = [hp.tile([128, N], F32, tag=f"pex{c}", name=f"pex{c}") for c in range(2)]
                for c, (o, p) in enumerate(NCH):
                    nmx = hp.tile([128, 1], F32, tag=f"nmx{c}", name=f"nmx{c}")
                    rs = hp.tile([128, 1], F32, tag=f"rs{c}", name=f"rs{c}")
                    nc.vector.tensor_reduce(out=nmx[:p, :], in_=acc[c][:p, :],
                                            op=OP.max, axis=mybir.AxisListType.X,
                                            negate=True)
                    nc.scalar.activation(pex[c][:p, :], acc[c][:p, :], AF.Exp,
                                         bias=nmx[:p, :], scale=1.0,
                                         accum_out=rs[:p, :])
                    nc.vector.reciprocal(rs[:p, :], rs[:p, :])
                    nc.vector.tensor_scalar(out=pex[c][:p, :], in0=pex[c][:p, :],
                                            scalar1=rs[:p, :], scalar2=None,
                                            op0=OP.mult)
                # transpose attn -> pT (j-part, i-free)
                pT = [hp.tile([128, N], F32, tag=f"pT{c}", name=f"pT{c}") for c in range(2)]
                for cj, (oj, pj) in enumerate(NCH):
                    for ci, (oi, pi) in enumerate(NCH):
                        tp = ps2.tile([128, 128], F32, tag="tp", name="tpa", bufs=2)
                        nc.tensor.transpose(tp[:pj, :pi],
                                            pex[ci][:pi, oj:oj + pj],
                                            ident[0:pi, 0:pi])
                        nc.scalar.copy(pT[cj][:pj, oi:oi + pi], tp[:pj, :pi])
                # attn @ v -> avT rows [32hs:32hs+16] of head-tile ht
                pav = ps.tile([128, N], F32, tag="mm", name="pav", bufs=3)
                for cj, (oj, pj) in enumerate(NCH):
                    nc.tensor.matmul(pav[32 * hs:32 * hs + 16, :],
                                     v_sb[cj][:pj, 16 * h:16 * h + 16],
                                     pT[cj][:pj, :],
                                     start=(cj == 0), stop=(cj == 1),
                                     tile_position=(0, 32 * hs))
                nc.scalar.copy(avT[ht][32 * hs:32 * hs + 16, :],
                               pav[32 * hs:32 * hs + 16, :])

            # -- combine + residual + instance norm --
            def inorm(xsum, ec, gname, bname, out_tile):
                """out = IN(xsum) with affine g,b. xsum: SBUF (128,N)."""
                sm = sp.tile([128, 1], F32, tag="sm", name="sm")
                sq = sp.tile([128, 1], F32, tag="sq", name="sq")
                sqd = sp.tile([128, N], F32, tag="sqd", name="sqd")
                nc.vector.tensor_reduce(out=sm[:], in_=xsum[:], op=OP.add,
                                        axis=mybir.AxisListType.X)
                nc.scalar.activation(sqd[:], xsum[:], AF.Square, accum_out=sq[:])
                mu = sp.tile([128, 1], F32, tag="mu", name="mu")
                var = sp.tile([128, 1], F32, tag="var", name="var")
                nc.vector.tensor_scalar(out=mu[:], in0=sm[:], scalar1=1.0 / N,
                                        scalar2=None, op0=OP.mult)
                # var = sq/N - mu^2
                musq = sp.tile([128, 1], F32, tag="musq", name="musq")
                nc.vector.tensor_tensor(out=musq[:], in0=mu[:], in1=mu[:], op=OP.mult)
                nc.vector.scalar_tensor_tensor(out=var[:], in0=sq[:], scalar=1.0 / N,
                                               in1=musq[:], op0=OP.mult, op1=OP.subtract)
                std = sp.tile([128, 1], F32, tag="std", name="std")
                nc.scalar.activation(std[:], var[:], AF.Sqrt, bias=eps_col)
                nc.vector.reciprocal(std[:], std[:])
                gs = sp.tile([128, 1], F32, tag="gs", name="gs")
                nc.vector.tensor_tensor(out=gs[:], in0=std[:], in1=ccol(gname, ec),
                                        op=OP.mult)
                nc.vector.tensor_scalar(out=out_tile[:], in0=xsum[:], scalar1=mu[:],
                                        scalar2=gs[:], op0=OP.subtract, op1=OP.mult)
                nc.scalar.activation(out_tile[:], out_tile[:], AF.Identity,
                                     bias=ccol(bname, ec), scale=1.0)

            o1 = [sp.tile([128, N], F32, tag=f"o1{ec}", name=f"o1{ec}") for ec in range(2)]
            for ec in range(2):
                pm = ps.tile([128, N], F32, tag="mm", name="pm", bufs=3)
                for m in range(4):
                    nc.tensor.matmul(pm[:], wc_sb[m][:, 128 * ec:128 * ec + 128],
                                     avT[m][:], start=(m == 0), stop=(m == 3))
                # x + mh + bc -> IN with g1,b1
                xs = sp.tile([128, N], F32, tag="xs", name="xs")
                nc.vector.scalar_tensor_tensor(out=xs[:], in0=pm[:],
                                               scalar=ccol('bc', ec), in1=xq[ec][:],
                                               op0=OP.add, op1=OP.add)
                inorm(xs, ec, 'g1', 'b1', o1[ec])

            # -- FFN --
            h1 = [sp.tile([128, N], F32, tag=f"h1{m}", name=f"h1{m}") for m in range(4)]
            for m in range(4):
                ph = ps.tile([128, N], F32, tag="mm", name="ph", bufs=3)
                for ec in range(2):
                    nc.tensor.matmul(ph[:], w1_sb[ec][:, 128 * m:128 * m + 128],
                                     o1[ec][:], start=(ec == 0), stop=(ec == 1))
                nc.scalar.activation(h1[m][:], ph[:], AF.Relu,
                                     bias=bw1_sb[:, m:m + 1], scale=1.0)
            xnew = [xp.tile([128, N], F32, tag=f"xn{s}{ec}", name=f"xn{s}{ec}") for ec in range(2)]
            for ec in range(2):
                po = ps.tile([128, N], F32, tag="mm", name="po", bufs=3)
                for m in range(4):
                    nc.tensor.matmul(po[:], w2_sb[m][:, 128 * ec:128 * ec + 128],
                                     h1[m][:], start=(m == 0), stop=(m == 3))
                xs2 = sp.tile([128, N], F32, tag="xs2", name="xs2")
                nc.vector.scalar_tensor_tensor(out=xs2[:], in0=po[:],
                                               scalar=ccol('bw2', ec), in1=o1[ec][:],
                                               op0=OP.add, op1=OP.add)
                inorm(xs2, ec, 'g2', 'b2', xnew[ec])
            return xnew

        for l in range(nlayers):
            xrow_new = block(l, 0, xrow, xcol, st_)
            xcol_new = block(l, 1, xcol, xrow, sT_)
            xrow, xcol = xrow_new, xcol_new

        for ec in range(2):
            nc.sync.dma_start(d_orow[128 * ec:128 * ec + 128, :], xrow[ec][:])
            nc.sync.dma_start(d_ocol[128 * ec:128 * ec + 128, :], xcol[ec][:])

    nc.compile()
    return nc


LAST_EXEC_NS = [None]


def make_runner(inputs):
    """Build the bass program + a reusable jitted multi-core executor.

    Returns run() -> (row, col) full-batch outputs."""
    import jax
    from jax.sharding import Mesh, PartitionSpec
    from jax.experimental.shard_map import shard_map
    from concourse import bass2jax, mybir as _mb

    w = _prep_weights(inputs)
    mlpc = _mlp_consts(inputs)
    nc = build(w, mlpc)
    data = np.asarray(inputs['data'], np.float32)
    nr = np.asarray(inputs['node_rand'], np.float32)
    in_maps = []
    for b in range(B):
        m = {"data": data[b], "nr": nr[b].reshape(1, N)}
        m.update(w)
        in_maps.append(m)

    bass2jax.install_neuronx_cc_hook()
    partition_name = nc.partition_id_tensor.name if nc.partition_id_tensor else None
    in_names, out_names, out_avals, zero_outs = [], [], [], []
    for alloc in nc.m.functions[0].allocations:
        if not isinstance(alloc, _mb.MemoryLocationSet):
            continue
        name = alloc.memorylocations[0].name
        if alloc.kind == "ExternalInput":
            if name != partition_name:
                in_names.append(name)
        elif alloc.kind == "ExternalOutput":
            shape = tuple(alloc.tensor_shape)
            dtype = _mb.dt.np(alloc.dtype)
            out_names.append(name)
            out_avals.append(jax.core.ShapedArray(shape, dtype))
            zero_outs.append(np.zeros(shape, dtype))
    n_params = len(in_names)
    n_outs = len(out_avals)
    all_names = in_names + out_names + ([partition_name] if partition_name else [])
    donate = tuple(range(n_params, n_params + n_outs))

    def _body(*args):
        operands = list(args)
        if partition_name is not None:
            operands.append(bass2jax.partition_id_tensor())
        outs = bass2jax._bass_exec_p.bind(
            *operands, out_avals=tuple(out_avals), in_names=tuple(all_names),
            out_names=tuple(out_names), lowering_input_output_aliases=(),
            sim_require_finite=True, sim_require_nnan=True, nc=nc)
        return tuple(outs)

    devices = jax.devices()[:B]
    mesh = Mesh(np.asarray(devices), ("core",))
    sharded = jax.jit(
        shard_map(_body, mesh=mesh,
                  in_specs=(PartitionSpec("core"),) * (n_params + n_outs),
                  out_specs=(PartitionSpec("core"),) * n_outs,
                  check_rep=False),
        donate_argnums=donate, keep_unused=True)
    from jax.sharding import NamedSharding
    shd = NamedSharding(mesh, PartitionSpec("core"))
    concat_in = [jax.device_put(
        np.concatenate([np.asarray(in_maps[c][nm]) for c in range(B)], axis=0), shd)
        for nm in in_names]

    def run():
        zo = [np.concatenate([z] * B, axis=0) for z in zero_outs]
        outs = sharded(*concat_in, *zo)
        outs = [np.asarray(o) for o in outs]
        res = {}
        for i, nm in enumerate(out_names):
            per = np.split(outs[i], B, axis=0)
            res[nm] = per
        row = np.stack([res["orow"][b].T for b in range(B)])
        col = np.stack([res["ocol"][b].T for b in range(B)])
        return row.astype(np.float32), col.astype(np.float32)

    return run


def kernel(**inputs):
    run = make_runner(inputs)
    return run()


# revision 19
# speedup vs baseline: 354.1876x; 26.4214x over previous
"""Trainium2 Bass kernel for nn_Cross_Encoder (dense cross-transformer).

Sharding: data-parallel over batch B=8 across 8 NeuronCores (1 batch elem/core,
all params replicated). Exploits the rank-1 structure of the edge embedding:
edge_emb[b,i,j,:] = scaled[b,i,j]*edgeW + edgeB, so the relative-bias edge
score collapses to es[b,h,i,j] = scaled[i,j]*qw[h,i] + qb[h,i] with
qw = SCALE*q@edgeW_h, qb = SCALE*q@edgeB_h -- no (B,N,N,E) tensor is ever
materialized.
"""
import math
import numpy as np
from contextlib import ExitStack

import concourse.bass as bass
import concourse.tile as tile
from concourse import bacc, mybir
from concourse import bass_utils
from concourse.masks import make_identity

F32 = mybir.dt.float32
AF = mybir.ActivationFunctionType
OP = mybir.AluOpType

# Model dims (hardcoded per problem spec)
L, E, H, D, FF, MS = 5, 256, 16, 16, 512, 16
B, N = 8, 200
SCALE = 1.0 / math.sqrt(D)
EPS = 1e-5
NCH = [(0, 128), (128, 72)]  # i/j chunking of N=200 into partitions


def _prep_weights(inp):
    """Host-side reshapes/padding of weights into DRAM layouts."""
    w = {}
    g = lambda k: np.asarray(inp[k], dtype=np.float32)
    Wq, Wk, Wv, Wc = g('Wq'), g('Wk'), g('Wv'), g('Wc')
    W1, W2 = g('W1'), g('W2')
    edgeW, edgeB = g('edgeW'), g('edgeB')
    for l in range(L):
        for s in range(2):
            t = f"{l}{s}"
            wqp = np.zeros((E, 2 * E), np.float32)
            wkp = np.zeros((E, 2 * E), np.float32)
            for h in range(H):
                wqp[:, 32 * h:32 * h + 16] = Wq[l, s][:, 16 * h:16 * h + 16] * SCALE
                wkp[:, 32 * h:32 * h + 16] = Wk[l, s][:, 16 * h:16 * h + 16]
            w[f"wq{t}"] = wqp
            w[f"wk{t}"] = wkp
            w[f"wv{t}"] = Wv[l, s]
            wcp = np.zeros((2 * E, E), np.float32)
            for h in range(H):
                wcp[32 * h:32 * h + 16, :] = Wc[l, s][16 * h:16 * h + 16, :]
            w[f"wc{t}"] = wcp
            w[f"w1{t}"] = W1[l, s]
            w[f"w2{t}"] = W2[l, s]
            cols = {}
            cols['bc'] = g('bc')[l, s]
            cols['g1'] = g('g1')[l, s]
            cols['b1'] = g('b1')[l, s]
            cols['g2'] = g('g2')[l, s]
            cols['b2'] = g('b2')[l, s]
            cols['bw2'] = g('bW2')[l, s]
            w[f"cols{t}"] = np.stack([cols[k] for k in
                                      ('bc', 'g1', 'b1', 'g2', 'b2', 'bw2')], axis=1)  # (256,6)
            w[f"bw1{t}"] = g('bW1')[l, s].reshape(FF // 128, 128).T.copy()  # (128,4)
    # edgeW/edgeB block layout for qw/qb matmuls: (128, 32), per head-tile t:
    # cols [8t..8t+4) = edgeW for heads 4t..4t+4, cols [8t+4..8t+8) = edgeB
    ewb = np.zeros((128, 32), np.float32)
    for hh in range(H):
        t, s = hh // 4, hh % 4
        for d in range(16):
            ewb[32 * s + d, 8 * t + s] = edgeW[16 * hh + d]
            ewb[32 * s + d, 8 * t + 4 + s] = edgeB[16 * hh + d]
    w["ewb"] = ewb
    w["nodewb"] = np.stack([g('nodeW'), g('nodeB')], axis=1)  # (256,2)
    # activation-bias constants: gamma[l,s,h,m] grid + EPS (broadcast to rows)
    gam = np.asarray(inp['m1b'], np.float32).reshape(1, -1)  # (1, L*2*H*MS)
    w["gam"] = np.concatenate([gam, np.full((1, 1), EPS, np.float32)], axis=1)
    return w


def _mlp_consts(inp):
    """Per (l,s,h,m) scalar constants for the mixed-score MLP, baked as imms."""
    m1w = np.asarray(inp['m1w'], np.float64)  # (L,2,H,2,MS)
    m1b = np.asarray(inp['m1b'], np.float64)  # (L,2,H,MS)
    m2w = np.asarray(inp['m2w'], np.float64)  # (L,2,H,MS)
    m2b = np.asarray(inp['m2b'], np.float64)  # (L,2,H)
    return m1w, m1b, m2w, m2b


def build(w, mlpc, nlayers=L):
    m1w, m1b, m2w, m2b = mlpc
    nc = bacc.Bacc("TRN2", target_bir_lowering=False, debug=False, num_devices=8)

    # --- DRAM I/O ---
    d_data = nc.dram_tensor("data", [N, N], F32, kind="ExternalInput").ap()
    d_nr = nc.dram_tensor("nr", [1, N], F32, kind="ExternalInput").ap()
    dw = {k: nc.dram_tensor(k, list(v.shape), F32, kind="ExternalInput").ap()
          for k, v in w.items()}
    d_orow = nc.dram_tensor("orow", [E, N], F32, kind="ExternalOutput").ap()
    d_ocol = nc.dram_tensor("ocol", [E, N], F32, kind="ExternalOutput").ap()
    d_scr = nc.dram_tensor("scratch", [1, 2], F32).ap()  # min/max roundtrip

    with tile.TileContext(nc) as tc, ExitStack() as ctx:
        cst = ctx.enter_context(tc.tile_pool(name="cst", bufs=1))
        wp = ctx.enter_context(tc.tile_pool(name="wp", bufs=2))
        xp = ctx.enter_context(tc.tile_pool(name="xp", bufs=2))
        qk = ctx.enter_context(tc.tile_pool(name="qk", bufs=2))
        hp = ctx.enter_context(tc.tile_pool(name="hp", bufs=4))
        sp = ctx.enter_context(tc.tile_pool(name="sp", bufs=2))
        ps = ctx.enter_context(tc.tile_pool(name="ps", bufs=3, space="PSUM"))
        ps2 = ctx.enter_context(tc.tile_pool(name="ps2", bufs=2, space="PSUM"))

        ident = cst.tile([128, 128], F32)
        make_identity(nc, ident[:])

        # ---- load data, compute scaled = (data-mn)/rng, and its transpose ----
        dt_ = [cst.tile([128, N], F32, tag=f"dt{c}", name=f"dt{c}") for c in range(2)]
        for c, (o, p) in enumerate(NCH):
            nc.sync.dma_start(dt_[c][:p, :], d_data[o:o + p, :])
        # per-chunk row max/min -> combine columns -> transpose -> scalar
        ext = cst.tile([128, 4], F32)  # cols: mx c0, mx c1, mn c0, mn c1
        nc.vector.memset(ext[:, 0:2], -3.0e38)
        nc.vector.memset(ext[:, 2:4], 3.0e38)
        for c, (o, p) in enumerate(NCH):
            nc.vector.tensor_reduce(out=ext[:p, c:c + 1], in_=dt_[c][:p, :],
                                    op=OP.max, axis=mybir.AxisListType.X)
            nc.vector.tensor_reduce(out=ext[:p, 2 + c:3 + c], in_=dt_[c][:p, :],
                                    op=OP.min, axis=mybir.AxisListType.X)
        ext2 = cst.tile([128, 2], F32)  # col0 = per-part max, col1 = per-part min
        nc.vector.tensor_tensor(out=ext2[:, 0:1], in0=ext[:, 0:1], in1=ext[:, 1:2],
                                op=OP.max)
        nc.vector.tensor_tensor(out=ext2[:, 1:2], in0=ext[:, 2:3], in1=ext[:, 3:4],
                                op=OP.min)
        extT = cst.tile([1, 256], F32)  # [0:128]=max vals, [128:256]=min vals
        for c2 in range(2):
            tps0 = ps2.tile([1, 128], F32, tag="tp", name="tps0", bufs=2)
            nc.tensor.transpose(tps0[:], ext2[:, c2:c2 + 1], ident[:])
            nc.scalar.copy(extT[:, 128 * c2:128 * c2 + 128], tps0[:])
        fin = cst.tile([1, 2], F32)  # [mx, mn]
        nc.vector.tensor_reduce(out=fin[:, 0:1], in_=extT[:, 0:128], op=OP.max,
                                axis=mybir.AxisListType.X)
        nc.vector.tensor_reduce(out=fin[:, 1:2], in_=extT[:, 128:256], op=OP.min,
                                axis=mybir.AxisListType.X)
        nc.sync.dma_start(d_scr, fin[:])
        mxmn = cst.tile([128, 2], F32)
        nc.sync.dma_start(mxmn[:], d_scr[0:1, :].broadcast_to((128, 2)))
        rng = cst.tile([128, 1], F32)
        nc.vector.tensor_tensor(out=rng[:], in0=mxmn[:, 0:1], in1=mxmn[:, 1:2],
                                op=OP.subtract)
        rinv = cst.tile([128, 1], F32)
        nc.vector.reciprocal(rinv[:], rng[:])
        # s = (data - mn) * rinv ; per chunk
        st_ = [cst.tile([128, N], F32, tag=f"st{c}", name=f"st{c}") for c in range(2)]
        for c, (o, p) in enumerate(NCH):
            nc.vector.tensor_scalar(out=st_[c][:p, :], in0=dt_[c][:p, :],
                                    scalar1=mxmn[:p, 1:2], scalar2=rinv[:p, :],
                                    op0=OP.subtract, op1=OP.mult)
        # transposed scaled (for col blocks)
        sT_ = [cst.tile([128, N], F32, tag=f"sT{c}", name=f"sT{c}") for c in range(2)]
        for cj, (oj, pj) in enumerate(NCH):
            for ci, (oi, pi) in enumerate(NCH):
                tp = ps2.tile([128, 128], F32, tag="tp", name="tps", bufs=2)
                nc.tensor.transpose(tp[:pj, :pi], st_[ci][:pi, oj:oj + pj],
                                    ident[0:pi, 0:pi])
                nc.scalar.copy(sT_[cj][:pj, oi:oi + pi], tp[:pj, :pi])

        # ---- node embedding: xT[e,n] = nodeW[e]*nr[n] + nodeB[e] ----
        nrb = cst.tile([128, N], F32)
        nc.sync.dma_start(nrb[:], d_nr[0:1, :].broadcast_to((128, N)))
        nwb = cst.tile([128, 4], F32)  # [nodeW ec, nodeB ec] x 2 chunks
        nc.sync.dma_start(nwb[:, 0:2], dw["nodewb"][0:128, :])
        nc.sync.dma_start(nwb[:, 2:4], dw["nodewb"][128:256, :])
        xrow = [xp.tile([128, N], F32, tag=f"xr{ec}", name=f"xr{ec}") for ec in range(2)]
        xcol = [xp.tile([128, N], F32, tag=f"xc{ec}", name=f"xc{ec}") for ec in range(2)]
        for ec in range(2):
            nc.vector.tensor_scalar(out=xrow[ec][:], in0=nrb[:],
                                    scalar1=nwb[:, 2 * ec:2 * ec + 1],
                                    scalar2=nwb[:, 2 * ec + 1:2 * ec + 2],
                                    op0=OP.mult, op1=OP.add)
            nc.vector.tensor_copy(xcol[ec][:], xrow[ec][:])

        ewb_sb = cst.tile([128, 32], F32)
        nc.sync.dma_start(ewb_sb[:], dw["ewb"])
        ngam = w["gam"].shape[1]
        gam_sb = cst.tile([128, ngam], F32)
        nc.sync.dma_start(gam_sb[:], dw["gam"][0:1, :].broadcast_to((128, ngam)))
        eps_col = gam_sb[:, ngam - 1:ngam]

        # =================== transformer blocks ===================
        def block(l, s, xq, xkv, stiles):
            """One encoder block; returns new stream tiles (2x (128,N) e-chunks)."""
            t = f"{l}{s}"
            # -- stream weights in --
            wq_sb = [wp.tile([128, 512], F32, tag=f"wq{ec}", name=f"wq{ec}") for ec in range(2)]
            wk_sb = [wp.tile([128, 512], F32, tag=f"wk{ec}", name=f"wk{ec}") for ec in range(2)]
            wv_sb = [wp.tile([128, 256], F32, tag=f"wv{ec}", name=f"wv{ec}") for ec in range(2)]
            wc_sb = [wp.tile([128, 256], F32, tag=f"wc{m}", name=f"wc{m}") for m in range(4)]
            w1_sb = [wp.tile([128, 512], F32, tag=f"w1{ec}", name=f"w1{ec}") for ec in range(2)]
            w2_sb = [wp.tile([128, 256], F32, tag=f"w2{m}", name=f"w2{m}") for m in range(4)]
            for ec in range(2):
                nc.sync.dma_start(wq_sb[ec][:], dw[f"wq{t}"][128 * ec:128 * ec + 128, :])
                nc.sync.dma_start(wk_sb[ec][:], dw[f"wk{t}"][128 * ec:128 * ec + 128, :])
                nc.sync.dma_start(wv_sb[ec][:], dw[f"wv{t}"][128 * ec:128 * ec + 128, :])
                nc.sync.dma_start(w1_sb[ec][:], dw[f"w1{t}"][128 * ec:128 * ec + 128, :])
            for m in range(4):
                nc.sync.dma_start(wc_sb[m][:], dw[f"wc{t}"][128 * m:128 * m + 128, :])
                nc.sync.dma_start(w2_sb[m][:], dw[f"w2{t}"][128 * m:128 * m + 128, :])
            colw = wp.tile([128, 12], F32, tag="colw", name="colw")  # 6 cols x 2 e-chunks
            nc.sync.dma_start(colw[:, 0:6], dw[f"cols{t}"][0:128, :])
            nc.sync.dma_start(colw[:, 6:12], dw[f"cols{t}"][128:256, :])
            bw1_sb = wp.tile([128, 4], F32, tag="bw1", name="bw1")
            nc.sync.dma_start(bw1_sb[:], dw[f"bw1{t}"])

            def ccol(name, ec):
                i = ('bc', 'g1', 'b1', 'g2', 'b2', 'bw2').index(name)
                return colw[:, 6 * ec + i:6 * ec + i + 1]

            # -- q/k projections (padded head layout), v natural --
            qT = [qk.tile([128, N], F32, tag=f"qT{m}", name=f"qT{m}") for m in range(4)]
            kT = [qk.tile([128, N], F32, tag=f"kT{m}", name=f"kT{m}") for m in range(4)]
            for m in range(4):
                pq = ps.tile([128, N], F32, tag="mm", name="pq", bufs=4)
                pk = ps.tile([128, N], F32, tag="mm", name="pk", bufs=4)
                for ec in range(2):
                    nc.tensor.matmul(pq[:], wq_sb[ec][:, 128 * m:128 * m + 128],
                                     xq[ec][:], start=(ec == 0), stop=(ec == 1))
                for ec in range(2):
                    nc.tensor.matmul(pk[:], wk_sb[ec][:, 128 * m:128 * m + 128],
                                     xkv[ec][:], start=(ec == 0), stop=(ec == 1))
                nc.scalar.copy(qT[m][:], pq[:])
                nc.scalar.copy(kT[m][:], pk[:])
            v_sb = [qk.tile([128, 256], F32, tag=f"v{c}", name=f"v{c}") for c in range(2)]
            for c, (o, p) in enumerate(NCH):
                pv = ps.tile([128, 256], F32, tag="mm", name="pv", bufs=4)
                for ec in range(2):
                    nc.tensor.matmul(pv[:p, :], xkv[ec][:, o:o + p], wv_sb[ec][:],
                                     start=(ec == 0), stop=(ec == 1))
                nc.scalar.copy(v_sb[c][:p, :], pv[:p, :])

            # -- qw/qb: per head-tile m, per i-chunk --
            qwb = [qk.tile([128, 32], F32, tag=f"qwb{c}", name=f"qwb{c}") for c in range(2)]
            for c, (o, p) in enumerate(NCH):
                for m in range(4):
                    pw = ps2.tile([128, 8], F32, tag="pw", name="pw", bufs=2)
                    nc.tensor.matmul(pw[:p, :], qT[m][:, o:o + p],
                                     ewb_sb[:, 8 * m:8 * m + 8], start=True, stop=True)
                    nc.scalar.copy(qwb[c][:p, 8 * m:8 * m + 8], pw[:p, :])

            # -- per-head attention --
            avT = [hp.tile([128, N], F32, tag=f"avT{m}", name=f"avT{m}") for m in range(4)]
            for h in range(H):
                ht, hs = h // 4, h % 4
                dot = [hp.tile([128, N], F32, tag=f"dot{c}", name=f"dot{c}") for c in range(2)]
                es = [hp.tile([128, N], F32, tag=f"es{c}", name=f"es{c}") for c in range(2)]
                for c, (o, p) in enumerate(NCH):
                    pd = ps.tile([128, N], F32, tag="mm", name="pd", bufs=4)
                    nc.tensor.matmul(pd[:p, :],
                                     qT[ht][32 * hs:32 * hs + 16, o:o + p],
                                     kT[ht][32 * hs:32 * hs + 16, :],
                                     start=True, stop=True,
                                     tile_position=(32 * hs, 0))
                    nc.scalar.copy(dot[c][:p, :], pd[:p, :])
                    nc.vector.tensor_scalar(
                        out=es[c][:p, :], in0=stiles[c][:p, :],
                        scalar1=qwb[c][:p, 8 * ht + hs:8 * ht + hs + 1],
                        scalar2=qwb[c][:p, 8 * ht + 4 + hs:8 * ht + 5 + hs],
                        op0=OP.mult, op1=OP.add)
                # mixed-score MLP: acc = sum_m c_m relu(a_m*dot + b_m*es + g_m) + m2b
                acc = [hp.tile([128, N], F32, tag=f"acc{c}", name=f"acc{c}") for c in range(2)]
                tmp = [hp.tile([128, N], F32, tag=f"tmp{c}", name=f"tmp{c}") for c in range(2)]
                for c, (o, p) in enumerate(NCH):
                    nc.vector.memset(acc[c][:p, :], float(m2b[l, s, h]))
                for m in range(MS):
                    al = float(m1w[l, s, h, 0, m]); be = float(m1w[l, s, h, 1, m])
                    c2 = float(m2w[l, s, h, m])
                    gidx = ((l * 2 + s) * H + h) * MS + m
                    gcol = gam_sb[:, gidx:gidx + 1]
                    for c, (o, p) in enumerate(NCH):
                        if abs(al) >= 1e-6:
                            nc.vector.scalar_tensor_tensor(
                                out=tmp[c][:p, :], in0=es[c][:p, :],
                                scalar=be / abs(al), in1=dot[c][:p, :],
                                op0=OP.mult,
                                op1=(OP.add if al > 0 else OP.subtract))
                            nc.scalar.activation(tmp[c][:p, :], tmp[c][:p, :],
                                                 AF.Relu, bias=gcol[:p, :],
                                                 scale=abs(al))
                        else:
                            nc.vector.tensor_scalar(out=tmp[c][:p, :],
                                                    in0=es[c][:p, :], scalar1=be,
                                                    scalar2=None, op0=OP.mult)
                            nc.scalar.activation(tmp[c][:p, :], tmp[c][:p, :],
                                                 AF.Relu, bias=gcol[:p, :])
                        nc.vector.scalar_tensor_tensor(
                            out=acc[c][:p, :], in0=tmp[c][:p, :], scalar=c2,
                            in1=acc[c][:p, :], op0=OP.mult, op1=OP.add)
                # softmax over j (free axis), normalized before transpose
                pex = [hp.tile([128, N], F32, tag=f"pex{c}", name=f"pex{c}") for c in range(2)]
                for c, (o, p) in enumerate(NCH):
                    nmx = hp.tile([128, 1], F32, tag=f"nmx{c}", name=f"nmx{c}")
                    rs = hp.tile([128, 1], F32, tag=f"rs{c}", name=f"rs{c}")
                    nc.vector.tensor_reduce(out=nmx[:p, :], in_=acc[c][:p, :],
                                            op=OP.max, axis=mybir.AxisListType.X,
                                            negate=True)
                    nc.scalar.activation(pex[c][:p, :], acc[c][:p, :], AF.Exp,
                                         bias=nmx[:p, :], scale=1.0,
                                         accum_out=rs[:p, :])
                    nc.vector.reciprocal(rs[:p, :], rs[:p, :])
                    nc.vector.tensor_scalar(out=pex[c][:p, :], in0=pex[c][:p, :],
                                            scalar1=rs[:p, :], scalar2=None,
                                            op0=OP.mult)
                # transpose attn -> pT (j-part, i-free)
                pT = [hp.tile([128, N], F32, tag=f"pT{c}", name=f"pT{c}") for c in range(2)]
                for cj, (oj, pj) in enumerate(NCH):
                    for ci, (oi, pi) in enumerate(NCH):
                        tp = ps2.tile([128, 128], F32, tag="tp", name="tpa", bufs=2)
                        nc.tensor.transpose(tp[:pj, :pi],
                                            pex[ci][:pi, oj:oj + pj],
                                            ident[0:pi, 0:pi])
                        nc.scalar.copy(pT[cj][:pj, oi:oi + pi], tp[:pj, :pi])
                # attn @ v -> avT rows [32hs:32hs+16] of head-tile ht
                pav = ps.tile([128, N], F32, tag="mm", name="pav", bufs=4)
                for cj, (oj, pj) in enumerate(NCH):
                    nc.tensor.matmul(pav[32 * hs:32 * hs + 16, :],
                                     v_sb[cj][:pj, 16 * h:16 * h + 16],
                                     pT[cj][:pj, :],
                                     start=(cj == 0), stop=(cj == 1),
                                     tile_position=(0, 32 * hs))
                nc.scalar.copy(avT[ht][32 * hs:32 * hs + 16, :],
                               pav[32 * hs:32 * hs + 16, :])

            # -- combine + residual + instance norm --
            def inorm(xsum, ec, gname, bname, out_tile):
                """out = IN(xsum) with affine g,b. xsum: SBUF (128,N)."""
                sm = sp.tile([128, 1], F32, tag="sm", name="sm")
                sq = sp.tile([128, 1], F32, tag="sq", name="sq")
                sqd = sp.tile([128, N], F32, tag="sqd", name="sqd")
                nc.vector.tensor_reduce(out=sm[:], in_=xsum[:], op=OP.add,
                                        axis=mybir.AxisListType.X)
                nc.scalar.activation(sqd[:], xsum[:], AF.Square, accum_out=sq[:])
                mu = sp.tile([128, 1], F32, tag="mu", name="mu")
                var = sp.tile([128, 1], F32, tag="var", name="var")
                nc.vector.tensor_scalar(out=mu[:], in0=sm[:], scalar1=1.0 / N,
                                        scalar2=None, op0=OP.mult)
                # var = sq/N - mu^2
                musq = sp.tile([128, 1], F32, tag="musq", name="musq")
                nc.vector.tensor_tensor(out=musq[:], in0=mu[:], in1=mu[:], op=OP.mult)
                nc.vector.scalar_tensor_tensor(out=var[:], in0=sq[:], scalar=1.0 / N,
                                               in1=musq[:], op0=OP.mult, op1=OP.subtract)
                std = sp.tile([128, 1], F32, tag="std", name="std")
                nc.scalar.activation(std[:], var[:], AF.Sqrt, bias=eps_col)
                nc.vector.reciprocal(std[:], std[:])
                gs = sp.tile([128, 1], F32, tag="gs", name="gs")
                nc.vector.tensor_tensor(out=gs[:], in0=std[:], in1=ccol(gname, ec),
                                        op=OP.mult)
                nc.vector.tensor_scalar(out=out_tile[:], in0=xsum[:], scalar1=mu[:],
                                        scalar2=gs[:], op0=OP.subtract, op1=OP.mult)
                nc.scalar.activation(out_tile[:], out_tile[:], AF.Identity,
                                     bias=ccol(bname, ec), scale=1.0)

            o1 = [sp.tile([128, N], F32, tag=f"o1{ec}", name=f"o1{ec}") for ec in range(2)]
            for ec in range(2):
                pm = ps.tile([128, N], F32, tag="mm", name="pm", bufs=4)
                for m in range(4):
                    nc.tensor.matmul(pm[:], wc_sb[m][:, 128 * ec:128 * ec + 128],
                                     avT[m][:], start=(m == 0), stop=(m == 3))
                # x + mh + bc -> IN with g1,b1
                xs = sp.tile([128, N], F32, tag="xs", name="xs")
                nc.vector.scalar_tensor_tensor(out=xs[:], in0=pm[:],
                                               scalar=ccol('bc', ec), in1=xq[ec][:],
                                               op0=OP.add, op1=OP.add)
                inorm(xs, ec, 'g1', 'b1', o1[ec])

            # -- FFN --
            h1 = [sp.tile([128, N], F32, tag=f"h1{m}", name=f"h1{m}") for m in range(4)]
            for m in range(4):
                ph = ps.tile([128, N], F32, tag="mm", name="ph", bufs=4)
                for ec in range(2):
                    nc.tensor.matmul(ph[:], w1_sb[ec][:, 128 * m:128 * m + 128],
                                     o1[ec][:], start=(ec == 0), stop=(ec == 1))
                nc.scalar.activation(h1[m][:], ph[:], AF.Relu,
                                     bias=bw1_sb[:, m:m + 1], scale=1.0)
            xnew = [xp.tile([128, N], F32, tag=f"xn{s}{ec}", name=f"xn{s}{ec}") for ec in range(2)]
            for ec in range(2):
                po = ps.tile([128, N], F32, tag="mm", name="po", bufs=4)
                for m in range(4):
                    nc.tensor.matmul(po[:], w2_sb[m][:, 128 * ec:128 * ec + 128],
                                     h1[m][:], start=(m == 0), stop=(m == 3))
                xs2 = sp.tile([128, N], F32, tag="xs2", name="xs2")
                nc.vector.scalar_tensor_tensor(out=xs2[:], in0=po[:],
                                               scalar=ccol('bw2', ec), in1=o1[ec][:],
                                               op0=OP.add, op1=OP.add)
                inorm(xs2, ec, 'g2', 'b2', xnew[ec])
            return xnew

        for l in range(nlayers):
            xrow_new = block(l, 0, xrow, xcol, st_)
            xcol_new = block(l, 1, xcol, xrow, sT_)
            xrow, xcol = xrow_new, xcol_new

        for ec in range(2):
            nc.sync.dma_start(d_orow[128 * ec:128 * ec + 128, :], xrow[ec][:])
            nc.sync.dma_start(d_ocol[128 * ec:128 * ec + 128, :], xcol[ec][:])

    nc.compile()
    return nc


LAST_EXEC_NS = [None]


def make_runner(inputs, nlayers=L):
    """Build the bass program + a reusable jitted multi-core executor.

    Returns run() -> (row, col) full-batch outputs."""
    import jax
    from jax.sharding import Mesh, PartitionSpec
    from jax.experimental.shard_map import shard_map
    from concourse import bass2jax, mybir as _mb

    w = _prep_weights(inputs)
    mlpc = _mlp_consts(inputs)
    nc = build(w, mlpc, nlayers=nlayers)
    data = np.asarray(inputs['data'], np.float32)
    nr = np.asarray(inputs['node_rand'], np.float32)
    in_maps = []
    for b in range(B):
        m = {"data": data[b], "nr": nr[b].reshape(1, N)}
        m.update(w)
        in_maps.append(m)

    bass2jax.install_neuronx_cc_hook()
    partition_name = nc.partition_id_tensor.name if nc.partition_id_tensor else None
    in_names, out_names, out_avals, zero_outs = [], [], [], []
    for alloc in nc.m.functions[0].allocations:
        if not isinstance(alloc, _mb.MemoryLocationSet):
            continue
        name = alloc.memorylocations[0].name
        if alloc.kind == "ExternalInput":
            if name != partition_name:
                in_names.append(name)
        elif alloc.kind == "ExternalOutput":
            shape = tuple(alloc.tensor_shape)
            dtype = _mb.dt.np(alloc.dtype)
            out_names.append(name)
            out_avals.append(jax.core.ShapedArray(shape, dtype))
            zero_outs.append(np.zeros(shape, dtype))
    n_params = len(in_names)
    n_outs = len(out_avals)
    all_names = in_names + out_names + ([partition_name] if partition_name else [])
    donate = tuple(range(n_params, n_params + n_outs))

    def _body(*args):
        operands = list(args)
        if partition_name is not None:
            operands.append(bass2jax.partition_id_tensor())
        outs = bass2jax._bass_exec_p.bind(
            *operands, out_avals=tuple(out_avals), in_names=tuple(all_names),
            out_names=tuple(out_names), lowering_input_output_aliases=(),
            sim_require_finite=True, sim_require_nnan=True, nc=nc)
        return tuple(outs)

    devices = jax.devices()[:B]
    mesh = Mesh(np.asarray(devices), ("core",))
    sharded = jax.jit(
        shard_map(_body, mesh=mesh,
                  in_specs=(PartitionSpec("core"),) * (n_params + n_outs),
                  out_specs=(PartitionSpec("core"),) * n_outs,
                  check_rep=False),
        donate_argnums=donate, keep_unused=True)
    from jax.sharding import NamedSharding
    shd = NamedSharding(mesh, PartitionSpec("core"))
    concat_in = [jax.device_put(
        np.concatenate([np.asarray(in_maps[c][nm]) for c in range(B)], axis=0), shd)
        for nm in in_names]

    def run():
        zo = [np.concatenate([z] * B, axis=0) for z in zero_outs]
        outs = sharded(*concat_in, *zo)
        outs = [np.asarray(o) for o in outs]
        res = {}
        for i, nm in enumerate(out_names):
            per = np.split(outs[i], B, axis=0)
            res[nm] = per
        row = np.stack([res["orow"][b].T for b in range(B)])
        col = np.stack([res["ocol"][b].T for b in range(B)])
        return row.astype(np.float32), col.astype(np.float32)

    return run


def kernel(**inputs):
    run = make_runner(inputs)
    return run()
